# revision 1
# baseline (speedup 1.0000x reference)
"""Trainium2 Bass kernel for nn_Middle_Integ (subunit integrator network).

Fast path (valid for the graded inputs, verified at runtime):
  * hist kernel K_hist == 0  -> the lax.scan recurrence vanishes; all
    time steps decouple into elementwise ops.
  * ancestor-spike kernel is identical across all 128 subunits ->
    depthwise conv along time commutes with the C_den projection:
        filtered = conv(Z_pad, k0) @ C_den.T
    so  base = S_conv + theta_syn + (conv(Z_pad, k0) + Y) @ C_den.T.

The kernel shards the time dimension across 8 NeuronCores (2500 rows
each + 100-row halo for the causal conv).  Per core: whole-tensor DMA
loads (big transfers), then per 512-row group: conv as two batched
N=512 Toeplitz matmuls, G = Zc + Y (DVE), transpose G (PE),
G^T @ C_den^T (PE) -> base in PSUM, sigmoid/affine elementwise
(ACT + DVE) written straight into persistent SBUF output tensors,
stored back in three large DMAs per output.

Falls back to an exact numpy implementation if the fast-path
preconditions do not hold.
"""
import os
import sys

import numpy as np

for _p in ("/opt/trn_rl_repo", os.path.expanduser("~/.axon_site/_ro/trn_rl_repo")):
    if os.path.isdir(_p) and _p not in sys.path:
        sys.path.append(_p)

import ml_dtypes

T_DATA, S, T_HIST = 20000, 128, 100
NCORES = 8
TC = T_DATA // NCORES   # 2500 valid output rows per core
P = 128
NT = 20                 # padded output tiles per core (2560 rows)
NZ = NT + 1             # Z tiles per core (halo + pad -> 2688 rows)
NG = 5                  # groups of 4 tiles
BF16 = ml_dtypes.bfloat16

LAST_RESULTS = None     # BassKernelResults from the most recent run
_PROGRAM = None         # cached compiled Bass program


def _build_kern_np(delta, log_tau, K):
    """float32 mirror of reference._build_kern -> (S, T_HIST)."""
    delta = np.asarray(delta, np.float32)
    log_tau = np.asarray(log_tau, np.float32)
    K = np.asarray(K, np.float32)
    t = np.maximum(np.arange(T_HIST, dtype=np.float32)[None, :] - delta[:, None], 0.0)
    tt = t[:, :, None] / np.exp(log_tau)[None, None, :]
    return np.einsum('stb,sb->st', (tt * np.exp(-tt)).astype(np.float32), K)


def _build_program():
    import concourse.bacc as bacc
    import concourse.tile as tile
    from concourse import mybir

    dt = mybir.dt
    nc = bacc.Bacc("TRN2", target_bir_lowering=False, debug=False,
                   enable_asserts=False, num_devices=NCORES)

    CB4 = nc.dram_tensor("CB4", [P, 4, P], dt.bfloat16, kind="ExternalInput")
    ZH = nc.dram_tensor("ZH", [P, NZ, P], dt.bfloat16, kind="ExternalInput")
    # [:,0] = Y in (t,s) tiles; [:,1] = Sc'^T and [:,2] = (noise+theta_spike)^T in (s,t) tiles
    YSN = nc.dram_tensor("YSN", [P, 3, NT, P], dt.bfloat16, kind="ExternalInput")
    WRT = nc.dram_tensor("WRT", [P, 3, 4, P], dt.bfloat16, kind="ExternalInput")
    # outputs in (s,t) tiles: [:,0]=FY, [:,1]=MUZ, [:,2]=FZ
    OUT = nc.dram_tensor("OUT", [P, 3, NT, P], dt.bfloat16, kind="ExternalOutput")

    AF = mybir.ActivationFunctionType
    AL = mybir.AluOpType
    store_plan = {1: (0, 8), 3: (8, 16), 4: (16, 20)}

    with tile.TileContext(nc) as tc:
        with (
            tc.tile_pool(name="big", bufs=1) as bp,
            tc.tile_pool(name="work", bufs=4) as wp,
            tc.tile_pool(name="psumA", bufs=3, space="PSUM") as ppa,
            tc.tile_pool(name="psumB", bufs=3, space="PSUM") as ppb,
        ):
            zbig = bp.tile([P, NZ, P], dt.bfloat16, tag="zbig")
            ysn = bp.tile([P, 3, NT, P], dt.bfloat16, tag="ysn")
            cb = bp.tile([P, 4, P], dt.bfloat16, tag="cb")
            wrt = bp.tile([P, 3, 4, P], dt.bfloat16, tag="wrt")
            obig = bp.tile([P, 3, NT, P], dt.bfloat16, tag="obig")

            # ordered so each tensor lands just before its first consumer
            nc.sync.dma_start(cb[:], CB4[:])
            nc.sync.dma_start(zbig[:], ZH[:])
            nc.sync.dma_start(ysn[:, 0], YSN[:, 0])
            nc.sync.dma_start(ysn[:, 1], YSN[:, 1])
            nc.sync.dma_start(wrt[:], WRT[:])
            nc.sync.dma_start(ysn[:, 2], YSN[:, 2])

            cdt = cb[:, 0, :]
            w1 = cb[:, 1, :]
            w2 = cb[:, 2, :]
            idn = cb[:, 3, :]
            wsub = wrt[:, 0]
            wspk = wrt[:, 1]
            thsp = wrt[:, 2]

            for g in range(NG):
                b0 = 4 * g
                sl = slice(b0, b0 + 4)
                # G^T = conv(Z)^T + Y^T directly in (s,t): Z tiles are the
                # stationary operand, Toeplitz factors stream; Y^T via
                # identity matmul opens the PSUM group
                zc = ppa.tile([P, 4, P], dt.float32, tag="zc")
                nc.tensor.matmul(zc[:], idn, ysn[:, 0, sl, :],
                                 start=True, stop=False)
                for b in range(4):
                    nc.tensor.matmul(zc[:, b, :], zbig[:, b0 + b, :], w1,
                                     start=False, stop=False)
                    nc.tensor.matmul(zc[:, b, :], zbig[:, b0 + b + 1, :], w2,
                                     start=False, stop=(b == 3))

                # G^T -> bf16 SBUF
                gts = wp.tile([P, 4, P], dt.bfloat16, tag="gts")
                if g % 2 == 0:
                    nc.scalar.activation(gts[:], zc[:], AF.Copy)
                else:
                    nc.vector.tensor_copy(gts[:], zc[:])

                # base^T (s,t) = Sc'^T + C_den @ G^T : identity + one matmul
                bps = ppb.tile([P, 4, P], dt.float32, tag="bps")
                nc.tensor.matmul(bps[:], idn, ysn[:, 1, sl, :],
                                 start=True, stop=False)
                nc.tensor.matmul(bps[:], cdt, gts[:],
                                 start=False, stop=True)

                # x^T = sigmoid(base^T)  (bf16)
                x = wp.tile([P, 4, P], dt.bfloat16, tag="x")
                nc.scalar.activation(x[:], bps[:], AF.Sigmoid)

                # per-subunit affines: replicated bf16 tiles, all-SBUF DVE
                nc.vector.tensor_mul(obig[:, 0, sl, :], x[:], wsub)
                t1 = wp.tile([P, 4, P], dt.bfloat16, tag="t1")
                nc.vector.tensor_mul(t1[:], x[:], wspk)
                nc.vector.tensor_add(obig[:, 1, sl, :], t1[:], thsp)
                za = wp.tile([P, 4, P], dt.bfloat16, tag="za")
                nc.gpsimd.tensor_add(za[:], t1[:], ysn[:, 2, sl, :])
                nc.scalar.activation(obig[:, 2, sl, :], za[:], AF.Sigmoid)

                if g in store_plan:
                    lo, hi = store_plan[g]
                    nc.sync.dma_start(OUT[:, :, lo:hi, :], obig[:, :, lo:hi, :])

    nc.compile()
    return nc


def _tile_rows(arr, ntiles):
    """(ntiles*P, S) -> contiguous (P, ntiles, S): partition-major tiling."""
    a = arr.reshape(ntiles, P, arr.shape[1]).transpose(1, 0, 2)
    return np.ascontiguousarray(a)


def _untile_rows(arr):
    """(P, ntiles, S) -> (ntiles*P, S)."""
    return arr.transpose(1, 0, 2).reshape(-1, arr.shape[2])


def _prepare_in_maps(inputs, k0):
    Z = np.asarray(inputs['Z_ancest'], np.float32)
    Y = np.asarray(inputs['Y_ancest'], np.float32)
    Scv = np.asarray(inputs['S_conv'], np.float32) + \
        np.asarray(inputs['theta_syn'], np.float32)[None, :]
    Nv = np.asarray(inputs['noise'], np.float32)
    C = np.asarray(inputs['C_den'], np.float32)

    # static conv Toeplitz factors: W1T[i,t] = k0[t+99-i], W2T[i,t] = k0[t-29-i]
    ii = np.arange(P)[:, None]
    tt = np.arange(P)[None, :]
    k0p = np.zeros(256, np.float32)
    k0p[:T_HIST] = k0
    j1 = tt + (T_HIST - 1) - ii
    j2 = tt - (P - T_HIST + 1) - ii
    W1 = np.where((j1 >= 0) & (j1 < T_HIST), k0p[np.clip(j1, 0, 255)], 0.0).astype(np.float32)
    W2 = np.where((j2 >= 0) & (j2 < T_HIST), k0p[np.clip(j2, 0, 255)], 0.0).astype(np.float32)

    CdT = np.ascontiguousarray(C.T).astype(BF16)
    CB4 = np.ascontiguousarray(
        np.stack([CdT, W1.astype(BF16), W2.astype(BF16),
                  np.eye(P, dtype=BF16)], axis=1))
    # per-subunit params replicated along free dim, (s,t) layout, bf16
    repT = lambda v: np.broadcast_to(
        np.asarray(v, np.float32)[:, None, None], (P, 4, P)).astype(BF16)
    WRT = np.ascontiguousarray(np.stack(
        [repT(inputs['W_sub']), repT(inputs['W_spike']),
         repT(inputs['theta_spike'])], axis=1))

    Zext = np.concatenate([np.zeros((T_HIST, S), np.float32), Z,
                           np.zeros((NZ * P - TC - T_HIST, S), np.float32)], axis=0)
    Zext = Zext.astype(BF16)
    pad = NT * P - TC
    Nsp = Nv + np.asarray(inputs['theta_spike'], np.float32)[None, :]
    Yext = np.concatenate([Y, np.zeros((pad, S), np.float32)], axis=0).astype(BF16)
    Sext = np.concatenate([Scv, np.zeros((pad, S), np.float32)], axis=0).astype(BF16)
    Next = np.concatenate([Nsp, np.zeros((pad, S), np.float32)], axis=0).astype(BF16)

    in_maps = []
    for c in range(NCORES):
        t0 = TC * c
        zr = np.zeros((NZ * P, S), BF16)
        lo, hi = t0, min(t0 + NZ * P, Zext.shape[0])
        zr[:hi - lo] = Zext[lo:hi]
        lo, hi = t0, t0 + NT * P
        tr = lambda a: a.reshape(NT, P, S).transpose(2, 0, 1)
        ysn = np.ascontiguousarray(np.stack(
            [tr(Yext[lo:hi]), tr(Sext[lo:hi]),
             tr(Next[lo:hi])], axis=1))
        in_maps.append({
            "ZH": _tile_rows(zr, NZ), "YSN": ysn,
            "CB4": CB4, "WRT": WRT,
        })
    return in_maps


def _fast_path(inputs, k0):
    global LAST_RESULTS, _PROGRAM
    from concourse import bass_utils

    in_maps = _prepare_in_maps(inputs, k0)

    if _PROGRAM is None:
        _PROGRAM = _build_program()
    nc = _PROGRAM

    trace = bool(os.environ.get("KERNEL_TRACE"))
    res = bass_utils.run_bass_kernel_spmd(
        nc, in_maps, core_ids=list(range(NCORES)), trace=trace)
    LAST_RESULTS = res

    fys, fzs, muzs = [], [], []
    untr = lambda a: a.transpose(1, 2, 0).reshape(NT * P, S)
    for c in range(NCORES):
        o = np.asarray(res.results[c]["OUT"], np.float32)
        fys.append(untr(o[:, 0])[:TC])
        muzs.append(untr(o[:, 1])[:TC])
        fzs.append(untr(o[:, 2])[:TC])
    fy = np.concatenate(fys, axis=0)
    fz = np.concatenate(fzs, axis=0)
    muz = np.concatenate(muzs, axis=0)
    return fy, fz, muz, muz


def _fallback_numpy(inputs, hist_kf, anc_k):
    """Exact numpy mirror of the reference (handles the general case)."""
    Z = np.asarray(inputs['Z_ancest'], np.float32)
    Y = np.asarray(inputs['Y_ancest'], np.float32)
    Scv = np.asarray(inputs['S_conv'], np.float32)
    Nv = np.asarray(inputs['noise'], np.float32)
    C = np.asarray(inputs['C_den'], np.float32)
    th_syn = np.asarray(inputs['theta_syn'], np.float32)
    W_sub = np.asarray(inputs['W_sub'], np.float32)
    W_spk = np.asarray(inputs['W_spike'], np.float32)
    th_spk = np.asarray(inputs['theta_spike'], np.float32)

    hist_kf = hist_kf[:, ::-1]
    anc_kf = anc_k[:, ::-1]

    Zpad = np.concatenate([np.zeros((T_HIST, S), np.float32), Z], axis=0)
    A = Zpad @ C.T
    filt = np.zeros((T_DATA, S), np.float32)
    for i in range(T_HIST):
        filt += A[i:i + T_DATA] * anc_kf[:, i][None, :]
    base = Scv + th_syn[None, :] + filt + Y @ C.T

    def sig(v):
        with np.errstate(over='ignore'):
            return 1.0 / (1.0 + np.exp(-v))

    buf = np.zeros((S, T_HIST), np.float32)
    fy = np.empty((T_DATA, S), np.float32)
    fz = np.empty((T_DATA, S), np.float32)
    muz = np.empty((T_DATA, S), np.float32)
    for t in range(T_DATA):
        fh = np.einsum('st,st->s', buf, hist_kf)
        x = sig(base[t] + fh)
        down = x * W_spk + th_spk
        z = sig(down + Nv[t])
        buf[:, :-1] = buf[:, 1:]
        buf[:, -1] = z
        fy[t] = x * W_sub
        fz[t] = z
        muz[t] = down
    return fy, fz, muz, muz


def kernel(**inputs):
    hist_kf = _build_kern_np(inputs['delta_hist'], inputs['tau_hist'], inputs['K_hist'])
    anc_k = _build_kern_np(inputs['delta_spike'], inputs['tau_spike'], inputs['K_spike'])
    shared = np.allclose(anc_k, anc_k[0:1], rtol=1e-6, atol=1e-12)
    no_hist = np.all(hist_kf == 0.0)
    if shared and no_hist:
        return _fast_path(inputs, anc_k[0])
    return _fallback_numpy(inputs, hist_kf, anc_k)



# revision 14
# speedup vs baseline: 1.2432x; 1.2432x over previous
"""Trainium2 Bass kernel for nn_Middle_Integ (subunit integrator network).

Fast path (valid for the graded inputs, verified at runtime):
  * hist kernel K_hist == 0  -> the lax.scan recurrence vanishes; all
    time steps decouple into elementwise ops.
  * ancestor-spike kernel is identical across all 128 subunits ->
    depthwise conv along time commutes with the C_den projection:
        base = Sc' + (conv(Z, k0) + Y) @ C_den.T
    x  = sigmoid(base);  fz = sigmoid(W_spike*x + theta_spike + noise)
  * fy = W_sub*x and muz = W_spike*x + theta_spike are per-subunit
    affine relabelings of x -> reconstructed on the host from x.

Device kernel (time dim sharded across 8 cores, 2500 rows each):
  per group of 4 row-tiles (512 time steps, one PSUM bank):
    conv as 5 Toeplitz matmuls (Z tiles stationary in fp8, merged
    [W2|W1] moving operands), +Y^T on DVE while converting PSUM->bf16,
    C_den^T stationary matmul (bf16), +Sc'^T on DVE, sigmoid on ACT,
    +noise''/W_spike on DVE, scaled sigmoid on ACT.
  PE warmup matmuls ramp the tensor-engine clock during input DMA;
  DMA issues are spread across engines to parallelize descriptor
  generation; inputs Z/Sc ship fp8 (error budget allows), Y ships
  bf16 pre-scaled by 256 so the fp8 conv scale folds into C_den.

Falls back to an exact numpy implementation if the fast-path
preconditions do not hold.
"""
import os
import sys

import numpy as np

for _p in ("/opt/trn_rl_repo", os.path.expanduser("~/.axon_site/_ro/trn_rl_repo")):
    if os.path.isdir(_p) and _p not in sys.path:
        sys.path.append(_p)

import ml_dtypes

T_DATA, S, T_HIST = 20000, 128, 100
NCORES = 8
TC = T_DATA // NCORES   # 2500 valid output rows per core
P = 128
NT = 20                 # padded output tiles per core (2560 rows)
NZ = NT + 1             # Z tiles per core (halo + pad -> 2688 rows)
NG = 5                  # groups of 4 tiles
BF16 = ml_dtypes.bfloat16
FP8 = ml_dtypes.float8_e4m3

ZSC = 16.0              # Z is shipped as fp8 * ZSC
WSC = 16.0              # Toeplitz factors shipped as fp8 * WSC
CSC = 1.0 / (ZSC * WSC)  # folded into C_den (and Y pre-scale 1/CSC)

NWARM = int(os.environ.get("KERNEL_NWARM", "12"))
MM_FP8 = os.environ.get("KERNEL_MM_FP8", "1") == "1"   # Z + Toeplitz in fp8
SC_FP8 = os.environ.get("KERNEL_SC_FP8", "1") == "1"   # Sc' in fp8 (DVE operand)
GPS_DMA = os.environ.get("KERNEL_GPS_DMA", "1") == "1"  # issue DMAs from gpsimd

LAST_RESULTS = None     # BassKernelResults from the most recent run
_PROGRAM = None         # cached compiled Bass program


def _build_kern_np(delta, log_tau, K):
    """float32 mirror of reference._build_kern -> (S, T_HIST)."""
    delta = np.asarray(delta, np.float32)
    log_tau = np.asarray(log_tau, np.float32)
    K = np.asarray(K, np.float32)
    t = np.maximum(np.arange(T_HIST, dtype=np.float32)[None, :] - delta[:, None], 0.0)
    tt = t[:, :, None] / np.exp(log_tau)[None, None, :]
    return np.einsum('stb,sb->st', (tt * np.exp(-tt)).astype(np.float32), K)


def _build_program():
    import concourse.bacc as bacc
    import concourse.tile as tile
    from concourse import mybir

    dt = mybir.dt
    nc = bacc.Bacc("TRN2", target_bir_lowering=False, debug=False,
                   enable_asserts=False, num_devices=NCORES)

    mmdt = dt.float8e4 if MM_FP8 else dt.bfloat16
    scdt = dt.float8e4 if SC_FP8 else dt.bfloat16
    CST = nc.dram_tensor("CST", [P, P], dt.bfloat16, kind="ExternalInput")
    W8 = nc.dram_tensor("W8", [P, 6, P], mmdt, kind="ExternalInput")
    WSP = nc.dram_tensor("WSP", [P, 1], dt.float32, kind="ExternalInput")
    ZF8 = nc.dram_tensor("ZF8", [P, NZ, P], mmdt, kind="ExternalInput")
    YT = nc.dram_tensor("YT", [P, NT, P], dt.bfloat16, kind="ExternalInput")
    SCT = nc.dram_tensor("SCT", [P, NT, P], scdt, kind="ExternalInput")
    NT2 = nc.dram_tensor("NT2", [P, NT, P], dt.bfloat16, kind="ExternalInput")
    XO = nc.dram_tensor("XO", [P, NT, P], dt.bfloat16, kind="ExternalOutput")
    FZ = nc.dram_tensor("FZ", [P, NT, P], dt.bfloat16, kind="ExternalOutput")

    AF = mybir.ActivationFunctionType
    AL = mybir.AluOpType

    with tile.TileContext(nc) as tc:
        with (
            tc.tile_pool(name="big", bufs=1) as bp,
            tc.tile_pool(name="work", bufs=6) as wp,
            tc.tile_pool(name="psumA", bufs=3, space="PSUM") as ppa,
            tc.tile_pool(name="psumB", bufs=2, space="PSUM") as ppb,
            tc.tile_pool(name="psumW", bufs=1, space="PSUM") as ppw,
        ):
            cst = bp.tile([P, P], dt.bfloat16, tag="cst")
            w8 = bp.tile([P, 6, P], mmdt, tag="w8")
            wsp = bp.tile([P, 1], dt.float32, tag="wsp")
            z8 = bp.tile([P, NZ, P], mmdt, tag="z8")
            yt = bp.tile([P, NT, P], dt.bfloat16, tag="yt")
            sct = bp.tile([P, NT, P], scdt, tag="sct")
            nt2 = bp.tile([P, NT, P], dt.bfloat16, tag="nt2")
            xo = bp.tile([P, NT, P], dt.bfloat16, tag="xo")
            fzo = bp.tile([P, NT, P], dt.bfloat16, tag="fzo")

            # parallel descriptor generation: spread issues across engines,
            # ordered so each tensor lands just before its first consumer
            eng3 = nc.gpsimd if GPS_DMA else nc.sync
            nc.sync.dma_start(cst[:], CST[:])
            nc.scalar.dma_start(w8[:], W8[:])
            eng3.dma_start(wsp[:], WSP[:])
            nc.sync.dma_start(z8[:], ZF8[:])
            nc.scalar.dma_start(yt[:], YT[:])
            eng3.dma_start(sct[:], SCT[:])
            eng3.dma_start(nt2[:], NT2[:])

            # PE clock warmup while input DMAs stream (only needs cst)
            if NWARM:
                wu = ppw.tile([P, P], dt.float32, tag="wu")
                for _ in range(NWARM):
                    nc.tensor.matmul(wu[:], cst[:], cst[:],
                                     start=True, stop=True)

            for g in range(NG):
                b0 = 4 * g
                # conv^T in (s,t): Z tiles stationary (fp8), Toeplitz
                # factors stream.  First matmul covers the whole bank
                # (W1 zero-padded to 512) so accumulation starts clean.
                zc = ppa.tile([P, 4, P], dt.float32, tag="zc")
                nc.tensor.matmul(zc[:], z8[:, b0, :], w8[:, 0:4, :],
                                 start=True, stop=False)
                for k in range(1, 4):
                    nc.tensor.matmul(zc[:, k - 1:k + 1, :], z8[:, b0 + k, :],
                                     w8[:, 4:6, :], start=False, stop=False)
                nc.tensor.matmul(zc[:, 3, :], z8[:, b0 + 4, :], w8[:, 4, :],
                                 start=False, stop=True)

                # G^T = conv^T + Y^T : PSUM -> bf16 SBUF with the add fused
                gts = wp.tile([P, 4, P], dt.bfloat16, tag="gts")
                nc.vector.tensor_tensor(gts[:], zc[:], yt[:, b0:b0 + 4, :], AL.add)

                # base^T = C' @ G^T (C' = C_den/256, stationary)
                bps = ppb.tile([P, 4, P], dt.float32, tag="bps")
                nc.tensor.matmul(bps[:], cst[:], gts[:], start=True, stop=True)

                # + Sc'^T, then sigmoid -> x straight into the out buffer
                bsum = wp.tile([P, 4, P], dt.bfloat16, tag="bsum")
                nc.vector.tensor_tensor(bsum[:], bps[:], sct[:, b0:b0 + 4, :], AL.add)
                nc.scalar.activation(xo[:, b0:b0 + 4, :], bsum[:], AF.Sigmoid)

                # fz = sigmoid(W_spike * (x + (noise+theta)/W_spike))
                za = wp.tile([P, 4, P], dt.bfloat16, tag="za")
                nc.vector.tensor_tensor(za[:], xo[:, b0:b0 + 4, :],
                                        nt2[:, b0:b0 + 4, :], AL.add)
                nc.scalar.activation(fzo[:, b0:b0 + 4, :], za[:], AF.Sigmoid,
                                     scale=wsp[:])

                if g == 1:
                    eng3.dma_start(XO[:, 0:8, :], xo[:, 0:8, :])
                    nc.sync.dma_start(FZ[:, 0:8, :], fzo[:, 0:8, :])
                elif g == 3:
                    eng3.dma_start(XO[:, 8:16, :], xo[:, 8:16, :])
                    nc.sync.dma_start(FZ[:, 8:16, :], fzo[:, 8:16, :])
                elif g == 4:
                    eng3.dma_start(XO[:, 16:20, :], xo[:, 16:20, :])
                    nc.sync.dma_start(FZ[:, 16:20, :], fzo[:, 16:20, :])

    nc.compile()
    return nc


def _tile_rows(arr, ntiles):
    """(ntiles*P, S) -> contiguous (P, ntiles, S): partition-major tiling."""
    a = arr.reshape(ntiles, P, arr.shape[1]).transpose(1, 0, 2)
    return np.ascontiguousarray(a)


def _prepare_in_maps(inputs, k0):
    Z = np.asarray(inputs['Z_ancest'], np.float32)
    Y = np.asarray(inputs['Y_ancest'], np.float32)
    Scv = np.asarray(inputs['S_conv'], np.float32) + \
        np.asarray(inputs['theta_syn'], np.float32)[None, :]
    Nv = np.asarray(inputs['noise'], np.float32)
    C = np.asarray(inputs['C_den'], np.float32)
    w_spk = np.asarray(inputs['W_spike'], np.float32)
    th_spk = np.asarray(inputs['theta_spike'], np.float32)

    # static conv Toeplitz factors: W1[i,t] = k0[t+99-i], W2[i,t] = k0[t-29-i]
    ii = np.arange(P)[:, None]
    tt = np.arange(P)[None, :]
    k0p = np.zeros(256, np.float32)
    k0p[:T_HIST] = k0
    j1 = tt + (T_HIST - 1) - ii
    j2 = tt - (P - T_HIST + 1) - ii
    W1 = np.where((j1 >= 0) & (j1 < T_HIST), k0p[np.clip(j1, 0, 255)], 0.0)
    W2 = np.where((j2 >= 0) & (j2 < T_HIST), k0p[np.clip(j2, 0, 255)], 0.0)

    mmdt = FP8 if MM_FP8 else BF16
    scdt = FP8 if SC_FP8 else BF16
    W8 = np.zeros((P, 6, P), np.float32)
    W8[:, 0] = W1 * WSC
    W8[:, 4] = W2 * WSC
    W8[:, 5] = W1 * WSC
    W8 = W8.astype(mmdt)
    CST = np.ascontiguousarray(C.T * CSC).astype(BF16)
    WSP = np.ascontiguousarray(w_spk[:, None])

    pad = NT * P - TC
    Zext = np.concatenate([np.zeros((T_HIST, S), np.float32), Z * ZSC,
                           np.zeros((NZ * P - TC - T_HIST, S), np.float32)],
                          axis=0).astype(mmdt)
    Ys = (Y / CSC).astype(np.float32)
    Npp = (Nv + th_spk[None, :]) / w_spk[None, :]

    def trt(a, lo):  # rows [lo, lo+2560) -> (P, NT, P) (s,t) tiles, zero-pad
        buf = np.zeros((NT * P, S), a.dtype)
        hi = min(lo + NT * P, T_DATA)
        buf[:hi - lo] = a[lo:hi]
        return np.ascontiguousarray(
            buf.reshape(NT, P, S).transpose(2, 0, 1))

    in_maps = []
    for c in range(NCORES):
        t0 = TC * c
        zr = np.zeros((NZ * P, S), mmdt)
        hi = min(t0 + NZ * P, Zext.shape[0])
        zr[:hi - t0] = Zext[t0:hi]
        in_maps.append({
            "CST": CST, "W8": W8, "WSP": WSP,
            "ZF8": _tile_rows(zr, NZ),
            "YT": trt(Ys, t0).astype(BF16),
            "SCT": trt(Scv, t0).astype(scdt),
            "NT2": trt(Npp, t0).astype(BF16),
        })
    return in_maps


def _fast_path(inputs, k0):
    global LAST_RESULTS, _PROGRAM
    from concourse import bass_utils

    in_maps = _prepare_in_maps(inputs, k0)

    if _PROGRAM is None:
        _PROGRAM = _build_program()
    nc = _PROGRAM

    trace = bool(os.environ.get("KERNEL_TRACE"))
    res = bass_utils.run_bass_kernel_spmd(
        nc, in_maps, core_ids=list(range(NCORES)), trace=trace)
    LAST_RESULTS = res

    w_sub = np.asarray(inputs['W_sub'], np.float32)
    w_spk = np.asarray(inputs['W_spike'], np.float32)
    th_spk = np.asarray(inputs['theta_spike'], np.float32)

    xs, fzs = [], []
    untr = lambda a: a.transpose(1, 2, 0).reshape(NT * P, S)
    for c in range(NCORES):
        r = res.results[c]
        xs.append(untr(np.asarray(r["XO"], np.float32))[:TC])
        fzs.append(untr(np.asarray(r["FZ"], np.float32))[:TC])
    x = np.concatenate(xs, axis=0)
    fz = np.concatenate(fzs, axis=0)
    fy = x * w_sub[None, :]
    muz = x * w_spk[None, :] + th_spk[None, :]
    return fy, fz, muz, muz


def _fallback_numpy(inputs, hist_kf, anc_k):
    """Exact numpy mirror of the reference (handles the general case)."""
    Z = np.asarray(inputs['Z_ancest'], np.float32)
    Y = np.asarray(inputs['Y_ancest'], np.float32)
    Scv = np.asarray(inputs['S_conv'], np.float32)
    Nv = np.asarray(inputs['noise'], np.float32)
    C = np.asarray(inputs['C_den'], np.float32)
    th_syn = np.asarray(inputs['theta_syn'], np.float32)
    W_sub = np.asarray(inputs['W_sub'], np.float32)
    W_spk = np.asarray(inputs['W_spike'], np.float32)
    th_spk = np.asarray(inputs['theta_spike'], np.float32)

    hist_kf = hist_kf[:, ::-1]
    anc_kf = anc_k[:, ::-1]

    Zpad = np.concatenate([np.zeros((T_HIST, S), np.float32), Z], axis=0)
    A = Zpad @ C.T
    filt = np.zeros((T_DATA, S), np.float32)
    for i in range(T_HIST):
        filt += A[i:i + T_DATA] * anc_kf[:, i][None, :]
    base = Scv + th_syn[None, :] + filt + Y @ C.T

    def sig(v):
        with np.errstate(over='ignore'):
            return 1.0 / (1.0 + np.exp(-v))

    buf = np.zeros((S, T_HIST), np.float32)
    fy = np.empty((T_DATA, S), np.float32)
    fz = np.empty((T_DATA, S), np.float32)
    muz = np.empty((T_DATA, S), np.float32)
    for t in range(T_DATA):
        fh = np.einsum('st,st->s', buf, hist_kf)
        x = sig(base[t] + fh)
        down = x * W_spk + th_spk
        z = sig(down + Nv[t])
        buf[:, :-1] = buf[:, 1:]
        buf[:, -1] = z
        fy[t] = x * W_sub
        fz[t] = z
        muz[t] = down
    return fy, fz, muz, muz


def kernel(**inputs):
    hist_kf = _build_kern_np(inputs['delta_hist'], inputs['tau_hist'], inputs['K_hist'])
    anc_k = _build_kern_np(inputs['delta_spike'], inputs['tau_spike'], inputs['K_spike'])
    shared = np.allclose(anc_k, anc_k[0:1], rtol=1e-6, atol=1e-12)
    no_hist = np.all(hist_kf == 0.0)
    w_spk = np.asarray(inputs['W_spike'], np.float32)
    ranges_ok = (
        np.min(np.abs(w_spk)) > 1e-3
        and np.max(np.abs(np.asarray(inputs['Z_ancest']))) * ZSC < 200.0
        and np.max(np.abs(np.asarray(inputs['S_conv']))
                   + np.abs(np.asarray(inputs['theta_syn']))[None, :]) < 200.0
        and np.max(np.abs(anc_k[0])) * WSC < 200.0
    )
    if shared and no_hist and ranges_ok:
        return _fast_path(inputs, anc_k[0])
    return _fallback_numpy(inputs, hist_kf, anc_k)


# revision 22
# speedup vs baseline: 1.2936x; 1.0406x over previous
"""Trainium2 Bass kernel for nn_Middle_Integ (subunit integrator network).

Fast path (valid for the graded inputs, verified at runtime):
  * hist kernel K_hist == 0  -> the lax.scan recurrence vanishes; all
    time steps decouple into elementwise ops.
  * ancestor-spike kernel is identical across all 128 subunits ->
    depthwise conv along time commutes with the C_den projection:
        base = Sc' + (conv(Z, k0) + Y) @ C_den.T
    x  = sigmoid(base);  fz = sigmoid(W_spike*x + theta_spike + noise)
  * fy = W_sub*x and muz = W_spike*x + theta_spike are per-subunit
    affine relabelings of x -> reconstructed on the host from x.

Device kernel (time dim sharded across 8 cores, 2500 rows each):
  per group of 4 row-tiles (512 time steps, one PSUM bank):
    conv as 5 Toeplitz matmuls (Z tiles stationary in fp8, merged
    [W2|W1] moving operands), +Y^T on DVE while converting PSUM->bf16,
    C_den^T stationary matmul (bf16), +Sc'^T on DVE, sigmoid on ACT,
    +noise''/W_spike on DVE, scaled sigmoid on ACT.
  PE warmup matmuls ramp the tensor-engine clock during input DMA;
  DMA issues are spread across engines to parallelize descriptor
  generation; inputs Z/Sc ship fp8 (error budget allows), Y ships
  bf16 pre-scaled by 256 so the fp8 conv scale folds into C_den.

Falls back to an exact numpy implementation if the fast-path
preconditions do not hold.
"""
import os
import sys

import numpy as np

for _p in ("/opt/trn_rl_repo", os.path.expanduser("~/.axon_site/_ro/trn_rl_repo")):
    if os.path.isdir(_p) and _p not in sys.path:
        sys.path.append(_p)

import ml_dtypes

T_DATA, S, T_HIST = 20000, 128, 100
NCORES = 8
TC = T_DATA // NCORES   # 2500 valid output rows per core
P = 128
NT = 20                 # padded output tiles per core (2560 rows)
NZ = NT + 1             # Z tiles per core (halo + pad -> 2688 rows)
NG = 5                  # groups of 4 tiles
BF16 = ml_dtypes.bfloat16
FP8 = ml_dtypes.float8_e4m3

ZSC = 8.0               # Z is shipped as fp8 * ZSC
WSC = 4.0               # Toeplitz factors shipped as fp8 * WSC
YSC = ZSC * WSC         # Y pre-scale (matches conv PSUM scale, /CSC on C_den)
CSC = 1.0 / YSC

NWARM = int(os.environ.get("KERNEL_NWARM", "10"))
MM_FP8 = os.environ.get("KERNEL_MM_FP8", "1") == "1"   # PE operands in fp8

LAST_RESULTS = None     # BassKernelResults from the most recent run
_PROGRAM = None         # cached compiled Bass program


def _build_kern_np(delta, log_tau, K):
    """float32 mirror of reference._build_kern -> (S, T_HIST)."""
    delta = np.asarray(delta, np.float32)
    log_tau = np.asarray(log_tau, np.float32)
    K = np.asarray(K, np.float32)
    t = np.maximum(np.arange(T_HIST, dtype=np.float32)[None, :] - delta[:, None], 0.0)
    tt = t[:, :, None] / np.exp(log_tau)[None, None, :]
    return np.einsum('stb,sb->st', (tt * np.exp(-tt)).astype(np.float32), K)


def _build_program():
    import concourse.bacc as bacc
    import concourse.tile as tile
    from concourse import mybir

    dt = mybir.dt
    nc = bacc.Bacc("TRN2", target_bir_lowering=False, debug=False,
                   enable_asserts=False, num_devices=NCORES)

    mmdt = dt.float8e4 if MM_FP8 else dt.bfloat16
    CST = nc.dram_tensor("CST", [P, P], dt.bfloat16, kind="ExternalInput")
    # [:,0]=W2*WSC, [:,1]=W1*WSC, [:,2]=identity
    W8 = nc.dram_tensor("W8", [P, 3, P], mmdt, kind="ExternalInput")
    WSP = nc.dram_tensor("WSP", [P, 1], dt.float32, kind="ExternalInput")
    ZF8 = nc.dram_tensor("ZF8", [P, NZ, P], mmdt, kind="ExternalInput")
    YT = nc.dram_tensor("YT", [P, NT, P], mmdt, kind="ExternalInput")
    SCT = nc.dram_tensor("SCT", [P, NT, P], mmdt, kind="ExternalInput")
    NT2 = nc.dram_tensor("NT2", [P, NT, P], dt.bfloat16, kind="ExternalInput")
    XO = nc.dram_tensor("XO", [P, NT, P], dt.bfloat16, kind="ExternalOutput")
    FZ = nc.dram_tensor("FZ", [P, NT, P], dt.bfloat16, kind="ExternalOutput")

    AF = mybir.ActivationFunctionType
    AL = mybir.AluOpType

    with tile.TileContext(nc) as tc:
        with (
            tc.tile_pool(name="big", bufs=1) as bp,
            tc.tile_pool(name="work", bufs=6) as wp,
            tc.tile_pool(name="psumA", bufs=3, space="PSUM") as ppa,
            tc.tile_pool(name="psumB", bufs=3, space="PSUM") as ppb,
            tc.tile_pool(name="psumW", bufs=1, space="PSUM") as ppw,
        ):
            cst = bp.tile([P, P], dt.bfloat16, tag="cst")
            w8 = bp.tile([P, 3, P], mmdt, tag="w8")
            wsp = bp.tile([P, 1], dt.float32, tag="wsp")
            z8 = bp.tile([P, NZ, P], mmdt, tag="z8")
            yt = bp.tile([P, NT, P], mmdt, tag="yt")
            sct = bp.tile([P, NT, P], mmdt, tag="sct")
            nt2 = bp.tile([P, NT, P], dt.bfloat16, tag="nt2")
            xo = bp.tile([P, NT, P], dt.bfloat16, tag="xo")
            fzo = bp.tile([P, NT, P], dt.bfloat16, tag="fzo")

            # spread DMA descriptor generation across sync+gpsimd (scalar
            # is the ACT engine and stays free for sigmoids); chunk the
            # big tensors so group 0/1 data lands first
            nc.sync.dma_start(cst[:], CST[:])
            nc.gpsimd.dma_start(w8[:], W8[:])
            nc.sync.dma_start(z8[:, 0:9, :], ZF8[:, 0:9, :])
            nc.gpsimd.dma_start(yt[:, 0:8, :], YT[:, 0:8, :])
            nc.sync.dma_start(z8[:, 9:NZ, :], ZF8[:, 9:NZ, :])
            nc.gpsimd.dma_start(yt[:, 8:NT, :], YT[:, 8:NT, :])
            nc.gpsimd.dma_start(sct[:], SCT[:])
            nc.gpsimd.dma_start(wsp[:], WSP[:])
            nc.gpsimd.dma_start(nt2[:], NT2[:])

            # PE clock warmup while input DMAs stream (only needs cst)
            if NWARM:
                wu = ppw.tile([P, P], dt.float32, tag="wu")
                for _ in range(NWARM):
                    nc.tensor.matmul(wu[:], cst[:], cst[:],
                                     start=True, stop=True)

            for g in range(NG):
                b0 = 4 * g
                # bank = YSC*(Y^T + conv^T) in (s,t): identity matmul seeds
                # the whole bank with Y^T, then Z tiles (stationary, fp8)
                # accumulate the conv with streamed Toeplitz factors
                zc = ppa.tile([P, 4, P], dt.float32, tag="zc")
                nc.tensor.matmul(zc[:], w8[:, 2, :], yt[:, b0:b0 + 4, :],
                                 start=True, stop=False)
                nc.tensor.matmul(zc[:, 0, :], z8[:, b0, :], w8[:, 1, :],
                                 start=False, stop=False)
                for k in range(1, 4):
                    nc.tensor.matmul(zc[:, k - 1:k + 1, :], z8[:, b0 + k, :],
                                     w8[:, 0:2, :], start=False, stop=False)
                nc.tensor.matmul(zc[:, 3, :], z8[:, b0 + 4, :], w8[:, 0, :],
                                 start=False, stop=True)

                # G^T -> bf16 SBUF
                gts = wp.tile([P, 4, P], dt.bfloat16, tag="gts")
                nc.vector.tensor_copy(gts[:], zc[:])

                # base^T = Sc'^T (identity seed) + C' @ G^T  (C' = C_den/YSC)
                bps = ppb.tile([P, 4, P], dt.float32, tag="bps")
                nc.tensor.matmul(bps[:], w8[:, 2, :], sct[:, b0:b0 + 4, :],
                                 start=True, stop=False)
                nc.tensor.matmul(bps[:], cst[:], gts[:], start=False, stop=True)

                # x = sigmoid(base) straight from PSUM into the out buffer
                nc.scalar.activation(xo[:, b0:b0 + 4, :], bps[:], AF.Sigmoid)

                # fz = sigmoid(W_spike * (x + (noise+theta)/W_spike))
                za = wp.tile([P, 4, P], dt.bfloat16, tag="za")
                nc.vector.tensor_tensor(za[:], xo[:, b0:b0 + 4, :],
                                        nt2[:, b0:b0 + 4, :], AL.add)
                nc.scalar.activation(fzo[:, b0:b0 + 4, :], za[:], AF.Sigmoid,
                                     scale=wsp[:])

                if g == 1:
                    nc.sync.dma_start(XO[:, 0:8, :], xo[:, 0:8, :])
                    nc.gpsimd.dma_start(FZ[:, 0:8, :], fzo[:, 0:8, :])
                elif g == 3:
                    nc.sync.dma_start(XO[:, 8:16, :], xo[:, 8:16, :])
                    nc.gpsimd.dma_start(FZ[:, 8:16, :], fzo[:, 8:16, :])
                elif g == 4:
                    nc.sync.dma_start(XO[:, 16:20, :], xo[:, 16:20, :])
                    nc.gpsimd.dma_start(FZ[:, 16:20, :], fzo[:, 16:20, :])

    nc.compile()
    return nc


def _tile_rows(arr, ntiles):
    """(ntiles*P, S) -> contiguous (P, ntiles, S): partition-major tiling."""
    a = arr.reshape(ntiles, P, arr.shape[1]).transpose(1, 0, 2)
    return np.ascontiguousarray(a)


def _prepare_in_maps(inputs, k0):
    Z = np.asarray(inputs['Z_ancest'], np.float32)
    Y = np.asarray(inputs['Y_ancest'], np.float32)
    Scv = np.asarray(inputs['S_conv'], np.float32) + \
        np.asarray(inputs['theta_syn'], np.float32)[None, :]
    Nv = np.asarray(inputs['noise'], np.float32)
    C = np.asarray(inputs['C_den'], np.float32)
    w_spk = np.asarray(inputs['W_spike'], np.float32)
    th_spk = np.asarray(inputs['theta_spike'], np.float32)

    # static conv Toeplitz factors: W1[i,t] = k0[t+99-i], W2[i,t] = k0[t-29-i]
    ii = np.arange(P)[:, None]
    tt = np.arange(P)[None, :]
    k0p = np.zeros(256, np.float32)
    k0p[:T_HIST] = k0
    j1 = tt + (T_HIST - 1) - ii
    j2 = tt - (P - T_HIST + 1) - ii
    W1 = np.where((j1 >= 0) & (j1 < T_HIST), k0p[np.clip(j1, 0, 255)], 0.0)
    W2 = np.where((j2 >= 0) & (j2 < T_HIST), k0p[np.clip(j2, 0, 255)], 0.0)

    mmdt = FP8 if MM_FP8 else BF16
    W8 = np.zeros((P, 3, P), np.float32)
    W8[:, 0] = W2 * WSC
    W8[:, 1] = W1 * WSC
    W8[:, 2] = np.eye(P, dtype=np.float32)
    W8 = W8.astype(mmdt)
    CST = np.ascontiguousarray(C.T * CSC).astype(BF16)
    WSP = np.ascontiguousarray(w_spk[:, None])

    pad = NT * P - TC
    Zext = np.concatenate([np.zeros((T_HIST, S), np.float32), Z * ZSC,
                           np.zeros((NZ * P - TC - T_HIST, S), np.float32)],
                          axis=0).astype(mmdt)
    Ys = (Y * YSC).astype(np.float32)
    Npp = (Nv + th_spk[None, :]) / w_spk[None, :]

    def trt(a, lo):  # rows [lo, lo+2560) -> (P, NT, P) (s,t) tiles, zero-pad
        buf = np.zeros((NT * P, S), a.dtype)
        hi = min(lo + NT * P, T_DATA)
        buf[:hi - lo] = a[lo:hi]
        return np.ascontiguousarray(
            buf.reshape(NT, P, S).transpose(2, 0, 1))

    in_maps = []
    for c in range(NCORES):
        t0 = TC * c
        zr = np.zeros((NZ * P, S), mmdt)
        hi = min(t0 + NZ * P, Zext.shape[0])
        zr[:hi - t0] = Zext[t0:hi]
        in_maps.append({
            "CST": CST, "W8": W8, "WSP": WSP,
            "ZF8": _tile_rows(zr, NZ),
            "YT": trt(Ys, t0).astype(mmdt),
            "SCT": trt(Scv, t0).astype(mmdt),
            "NT2": trt(Npp, t0).astype(BF16),
        })
    return in_maps


def _fast_path(inputs, k0):
    global LAST_RESULTS, _PROGRAM
    from concourse import bass_utils

    in_maps = _prepare_in_maps(inputs, k0)

    if _PROGRAM is None:
        _PROGRAM = _build_program()
    nc = _PROGRAM

    trace = bool(os.environ.get("KERNEL_TRACE"))
    res = bass_utils.run_bass_kernel_spmd(
        nc, in_maps, core_ids=list(range(NCORES)), trace=trace)
    LAST_RESULTS = res

    w_sub = np.asarray(inputs['W_sub'], np.float32)
    w_spk = np.asarray(inputs['W_spike'], np.float32)
    th_spk = np.asarray(inputs['theta_spike'], np.float32)

    xs, fzs = [], []
    untr = lambda a: a.transpose(1, 2, 0).reshape(NT * P, S)
    for c in range(NCORES):
        r = res.results[c]
        xs.append(untr(np.asarray(r["XO"], np.float32))[:TC])
        fzs.append(untr(np.asarray(r["FZ"], np.float32))[:TC])
    x = np.concatenate(xs, axis=0)
    fz = np.concatenate(fzs, axis=0)
    fy = x * w_sub[None, :]
    muz = x * w_spk[None, :] + th_spk[None, :]
    return fy, fz, muz, muz


def _fallback_numpy(inputs, hist_kf, anc_k):
    """Exact numpy mirror of the reference (handles the general case)."""
    Z = np.asarray(inputs['Z_ancest'], np.float32)
    Y = np.asarray(inputs['Y_ancest'], np.float32)
    Scv = np.asarray(inputs['S_conv'], np.float32)
    Nv = np.asarray(inputs['noise'], np.float32)
    C = np.asarray(inputs['C_den'], np.float32)
    th_syn = np.asarray(inputs['theta_syn'], np.float32)
    W_sub = np.asarray(inputs['W_sub'], np.float32)
    W_spk = np.asarray(inputs['W_spike'], np.float32)
    th_spk = np.asarray(inputs['theta_spike'], np.float32)

    hist_kf = hist_kf[:, ::-1]
    anc_kf = anc_k[:, ::-1]

    Zpad = np.concatenate([np.zeros((T_HIST, S), np.float32), Z], axis=0)
    A = Zpad @ C.T
    filt = np.zeros((T_DATA, S), np.float32)
    for i in range(T_HIST):
        filt += A[i:i + T_DATA] * anc_kf[:, i][None, :]
    base = Scv + th_syn[None, :] + filt + Y @ C.T

    def sig(v):
        with np.errstate(over='ignore'):
            return 1.0 / (1.0 + np.exp(-v))

    buf = np.zeros((S, T_HIST), np.float32)
    fy = np.empty((T_DATA, S), np.float32)
    fz = np.empty((T_DATA, S), np.float32)
    muz = np.empty((T_DATA, S), np.float32)
    for t in range(T_DATA):
        fh = np.einsum('st,st->s', buf, hist_kf)
        x = sig(base[t] + fh)
        down = x * W_spk + th_spk
        z = sig(down + Nv[t])
        buf[:, :-1] = buf[:, 1:]
        buf[:, -1] = z
        fy[t] = x * W_sub
        fz[t] = z
        muz[t] = down
    return fy, fz, muz, muz


def kernel(**inputs):
    hist_kf = _build_kern_np(inputs['delta_hist'], inputs['tau_hist'], inputs['K_hist'])
    anc_k = _build_kern_np(inputs['delta_spike'], inputs['tau_spike'], inputs['K_spike'])
    shared = np.allclose(anc_k, anc_k[0:1], rtol=1e-6, atol=1e-12)
    no_hist = np.all(hist_kf == 0.0)
    w_spk = np.asarray(inputs['W_spike'], np.float32)
    ranges_ok = (
        np.min(np.abs(w_spk)) > 1e-3
        and np.max(np.abs(np.asarray(inputs['Z_ancest']))) * ZSC < 230.0
        and np.max(np.abs(np.asarray(inputs['Y_ancest']))) * YSC < 230.0
        and np.max(np.abs(np.asarray(inputs['S_conv']))
                   + np.abs(np.asarray(inputs['theta_syn']))[None, :]) < 230.0
        and np.max(np.abs(anc_k[0])) * WSC < 230.0
    )
    if shared and no_hist and ranges_ok:
        return _fast_path(inputs, anc_k[0])
    return _fallback_numpy(inputs, hist_kf, anc_k)
